# revision 1
# baseline (speedup 1.0000x reference)
"""DDUSAAdapterBlock on 8 NeuronCores.

Sharding: 8 shards = (batch b in 0..3) x (sequence half h in 0..1).
Each core gets the FULL inputs (replicated; no collectives) and computes
one 512-query slice of one batch end-to-end:
  - self-attention (keys/values over the full 1024-token sequence of its batch)
  - relative-coordinate-bias MLP for its 512x1024 (query,key) pairs
  - cross-attention with that bias
  - ConvFFN, computed on a 544-row window (its 512 rows + one 32-token
    image-row halo on the inner side) so the 3x3 depthwise conv sees the
    needed neighbor rows; the halo row is discarded after the conv.
The host concatenates the 8 x (512, 768) slices back to (4, 1024, 768).
"""

import numpy as np

B, N, D, H, HD, FF, RB = 4, 1024, 768, 12, 64, 3072, 64
IMG = 32          # target_h == target_w == 32
HALF = N // 2     # 512 tokens per shard
WIN = HALF + IMG  # 544-token window (17 image rows)


def _block_jax(jnp, lax, jgelu, i, qs, sk, sv, qc, sc,
               sa_in_w, sa_in_b, sa_out_w, sa_out_b,
               ca_q_w, ca_q_b, ca_k_w, ca_k_b, ca_v_w, ca_v_b,
               ca_out_w, ca_out_b,
               rb_w1, rb_b1, rb_w2, rb_b2,
               fc1_w, fc1_b, dw_w, dw_b, fc2_w, fc2_b,
               ln1_g, ln1_b, ln2_g, ln2_b, ln3_g, ln3_b):
    b = i // 2
    h = i % 2
    start = h * (HALF - IMG)      # 0 or 480
    own = h * IMG                 # offset of own 512 rows inside the window

    def ln(x, g, bb):
        m = jnp.mean(x, -1, keepdims=True)
        v = jnp.mean((x - m) ** 2, -1, keepdims=True)
        return (x - m) * lax.rsqrt(v + 1e-5) * g + bb

    def mha(q, k, v, bias=None):
        qh = q.reshape(-1, H, HD)
        kh = k.reshape(-1, H, HD)
        vh = v.reshape(-1, H, HD)
        logits = jnp.einsum('qhd,khd->hqk', qh, kh) * (HD ** -0.5)
        if bias is not None:
            logits = logits + bias
        attn = jax_softmax(logits)
        out = jnp.einsum('hqk,khd->qhd', attn, vh)
        return out.reshape(-1, D)

    def jax_softmax(x):
        m = jnp.max(x, axis=-1, keepdims=True)
        e = jnp.exp(x - m)
        return e / jnp.sum(e, axis=-1, keepdims=True)

    x_full = lax.dynamic_index_in_dim(qs, b, 0, keepdims=False)      # (N, D)
    sk_b = lax.dynamic_index_in_dim(sk, b, 0, keepdims=False)
    sv_b = lax.dynamic_index_in_dim(sv, b, 0, keepdims=False)
    qc_b = lax.dynamic_index_in_dim(qc, b, 0, keepdims=False)
    sc_b = lax.dynamic_index_in_dim(sc, b, 0, keepdims=False)

    # --- self-attention: full-sequence K/V, window queries ---
    qn = ln(x_full, ln1_g, ln1_b)
    qkv = qn @ sa_in_w + sa_in_b                                      # (N, 3D)
    q_f, k_f, v_f = jnp.split(qkv, 3, axis=-1)
    q_w = lax.dynamic_slice_in_dim(q_f, start, WIN, 0)                # (WIN, D)
    x_w = lax.dynamic_slice_in_dim(x_full, start, WIN, 0)
    so = mha(q_w, k_f, v_f) @ sa_out_w + sa_out_b
    x_w = x_w + so

    # --- relative coordinate bias for (WIN x N) pairs ---
    qc_w = lax.dynamic_slice_in_dim(qc_b, start, WIN, 0)              # (WIN, 2)
    delta = qc_w[:, None, :] - sc_b[None, :, :]                       # (WIN, N, 2)
    dx = delta[..., 0:1]
    dy = delta[..., 1:2]
    r2 = dx * dx + dy * dy
    r = jnp.sqrt(r2 + 1e-8)
    geom = jnp.concatenate([dx, dy, r, r2], axis=-1)                  # (WIN, N, 4)
    hb = jgelu(geom @ rb_w1 + rb_b1)                                  # (WIN, N, RB)
    bias = hb @ rb_w2 + rb_b2                                         # (WIN, N, H)
    bias = jnp.transpose(bias, (2, 0, 1))                             # (H, WIN, N)

    # --- cross-attention ---
    qn2 = ln(x_w, ln2_g, ln2_b)
    cq = qn2 @ ca_q_w + ca_q_b
    ck = sk_b @ ca_k_w + ca_k_b
    cv = sv_b @ ca_v_w + ca_v_b
    co = mha(cq, ck, cv, bias=bias) @ ca_out_w + ca_out_b
    x_w = x_w + co

    # --- ConvFFN on the 17-image-row window ---
    hn = jgelu(ln(x_w, ln3_g, ln3_b) @ fc1_w + fc1_b)                 # (WIN, FF)
    h2d = jnp.transpose(hn.reshape(WIN // IMG, IMG, FF), (2, 0, 1))[None]  # (1,FF,17,32)
    h2d = lax.conv_general_dilated(h2d, dw_w, (1, 1), 'SAME',
                                   feature_group_count=FF,
                                   dimension_numbers=('NCHW', 'OIHW', 'NCHW'))
    h2d = h2d + dw_b[None, :, None, None]
    hn = jnp.transpose(h2d[0], (1, 2, 0)).reshape(WIN, FF)
    hn = jgelu(hn)
    ffn = hn @ fc2_w + fc2_b
    out_w = x_w + ffn                                                 # (WIN, D)
    return lax.dynamic_slice_in_dim(out_w, own, HALF, 0)              # (HALF, D)


def _run_pmap(inp):
    import jax
    import jax.numpy as jnp
    from jax import lax

    def jgelu(x):
        return jax.nn.gelu(x, approximate=False)

    arg_names = ['query_state', 'source_key', 'source_value', 'query_coords',
                 'source_coords',
                 'sa_in_w', 'sa_in_b', 'sa_out_w', 'sa_out_b',
                 'ca_q_w', 'ca_q_b', 'ca_k_w', 'ca_k_b', 'ca_v_w', 'ca_v_b',
                 'ca_out_w', 'ca_out_b',
                 'rb_w1', 'rb_b1', 'rb_w2', 'rb_b2',
                 'ffn_fc1_w', 'ffn_fc1_b', 'ffn_dw_w', 'ffn_dw_b',
                 'ffn_fc2_w', 'ffn_fc2_b',
                 'ln1_g', 'ln1_b', 'ln2_g', 'ln2_b', 'ln3_g', 'ln3_b']
    args = [np.asarray(inp[n], dtype=np.float32) for n in arg_names]

    def per_core(idx, *ws):
        return _block_jax(jnp, lax, jgelu, idx, *ws)

    fn = jax.pmap(per_core, axis_name='i',
                  in_axes=(0,) + (None,) * len(args), devices=jax.devices()[:8])
    out = fn(np.arange(8, dtype=np.int32), *args)     # (8, HALF, D)
    out = np.asarray(out)
    # shard i = (b, h): rows [h*512:(h+1)*512) of batch b
    return out.reshape(B, 2, HALF, D).reshape(B, N, D)


def _run_numpy(inp):
    """Pure-numpy fallback (host) — exact same math."""
    from math import erf
    f32 = np.float32

    def gelu(x):
        # exact gelu via erf, vectorized
        from scipy.special import erf as verf  # noqa
        return (x * 0.5 * (1.0 + verf(x / np.sqrt(2.0)))).astype(f32)

    try:
        from scipy.special import erf as _  # noqa
        _gelu = gelu
    except Exception:
        def _gelu(x):
            t = np.tanh(np.sqrt(2 / np.pi) * (x + 0.044715 * x ** 3))
            return (0.5 * x * (1 + t)).astype(f32)

    def ln(x, g, b):
        m = x.mean(-1, keepdims=True)
        v = ((x - m) ** 2).mean(-1, keepdims=True)
        return (x - m) / np.sqrt(v + 1e-5) * g + b

    def softmax(x):
        m = x.max(-1, keepdims=True)
        e = np.exp(x - m)
        return e / e.sum(-1, keepdims=True)

    def mha(q, k, v, bias=None):
        b = q.shape[0]
        qh = q.reshape(b, -1, H, HD)
        kh = k.reshape(b, -1, H, HD)
        vh = v.reshape(b, -1, H, HD)
        logits = np.einsum('bqhd,bkhd->bhqk', qh, kh) * (HD ** -0.5)
        if bias is not None:
            logits = logits + bias
        a = softmax(logits)
        out = np.einsum('bhqk,bkhd->bqhd', a, vh)
        return out.reshape(b, -1, D)

    x = np.asarray(inp['query_state'], f32)
    qn = ln(x, inp['ln1_g'], inp['ln1_b'])
    qkv = qn @ inp['sa_in_w'] + inp['sa_in_b']
    q, k, v = np.split(qkv, 3, axis=-1)
    x = x + mha(q, k, v) @ inp['sa_out_w'] + inp['sa_out_b']

    qc = np.asarray(inp['query_coords'], f32)
    sc = np.asarray(inp['source_coords'], f32)
    delta = qc[:, :, None, :] - sc[:, None, :, :]
    dx = delta[..., 0:1]; dy = delta[..., 1:2]
    r2 = dx * dx + dy * dy
    r = np.sqrt(r2 + 1e-8)
    geom = np.concatenate([dx, dy, r, r2], axis=-1)
    hb = _gelu(geom @ inp['rb_w1'] + inp['rb_b1'])
    bias = hb @ inp['rb_w2'] + inp['rb_b2']
    bias = np.transpose(bias, (0, 3, 1, 2))

    qn = ln(x, inp['ln2_g'], inp['ln2_b'])
    cq = qn @ inp['ca_q_w'] + inp['ca_q_b']
    ck = np.asarray(inp['source_key'], f32) @ inp['ca_k_w'] + inp['ca_k_b']
    cv = np.asarray(inp['source_value'], f32) @ inp['ca_v_w'] + inp['ca_v_b']
    x = x + mha(cq, ck, cv, bias=bias) @ inp['ca_out_w'] + inp['ca_out_b']

    hn = _gelu(ln(x, inp['ln3_g'], inp['ln3_b']) @ inp['ffn_fc1_w'] + inp['ffn_fc1_b'])
    h2d = np.transpose(hn.reshape(B, IMG, IMG, FF), (0, 3, 1, 2))
    # 3x3 depthwise SAME conv via shifted adds
    w = np.asarray(inp['ffn_dw_w'], f32)      # (FF, 1, 3, 3)
    pad = np.pad(h2d, ((0, 0), (0, 0), (1, 1), (1, 1)))
    out = np.zeros_like(h2d)
    for di in range(3):
        for dj in range(3):
            out += pad[:, :, di:di + IMG, dj:dj + IMG] * w[None, :, 0, di, dj, None, None]
    out = out + np.asarray(inp['ffn_dw_b'], f32)[None, :, None, None]
    hn = np.transpose(out, (0, 2, 3, 1)).reshape(B, N, FF)
    hn = _gelu(hn)
    return (x + hn @ inp['ffn_fc2_w'] + inp['ffn_fc2_b']).astype(f32)


def kernel(**inputs) -> np.ndarray:
    try:
        return _run_pmap(inputs).astype(np.float32)
    except Exception:
        return _run_numpy(inputs)


if __name__ == '__main__':
    rng = np.random.default_rng(0)
    demo = {
        'query_state': rng.standard_normal((B, N, D), dtype=np.float32),
        'source_key': rng.standard_normal((B, N, D), dtype=np.float32),
        'source_value': rng.standard_normal((B, N, D), dtype=np.float32),
        'query_coords': rng.random((B, N, 2), dtype=np.float32),
        'source_coords': rng.random((B, N, 2), dtype=np.float32),
        'sa_in_w': rng.standard_normal((D, 3 * D), dtype=np.float32) * 0.02,
        'sa_in_b': np.zeros(3 * D, np.float32),
        'sa_out_w': rng.standard_normal((D, D), dtype=np.float32) * 0.02,
        'sa_out_b': np.zeros(D, np.float32),
        'ca_q_w': rng.standard_normal((D, D), dtype=np.float32) * 0.02,
        'ca_q_b': np.zeros(D, np.float32),
        'ca_k_w': rng.standard_normal((D, D), dtype=np.float32) * 0.02,
        'ca_k_b': np.zeros(D, np.float32),
        'ca_v_w': rng.standard_normal((D, D), dtype=np.float32) * 0.02,
        'ca_v_b': np.zeros(D, np.float32),
        'ca_out_w': rng.standard_normal((D, D), dtype=np.float32) * 0.02,
        'ca_out_b': np.zeros(D, np.float32),
        'rb_w1': rng.standard_normal((4, RB), dtype=np.float32) * 0.1,
        'rb_b1': np.zeros(RB, np.float32),
        'rb_w2': rng.standard_normal((RB, H), dtype=np.float32) * 0.1,
        'rb_b2': np.zeros(H, np.float32),
        'ffn_fc1_w': rng.standard_normal((D, FF), dtype=np.float32) * 0.02,
        'ffn_fc1_b': np.zeros(FF, np.float32),
        'ffn_dw_w': rng.standard_normal((FF, 1, 3, 3), dtype=np.float32) * 0.1,
        'ffn_dw_b': np.zeros(FF, np.float32),
        'ffn_fc2_w': rng.standard_normal((FF, D), dtype=np.float32) * 0.02,
        'ffn_fc2_b': np.zeros(D, np.float32),
        'ln1_g': np.ones(D, np.float32), 'ln1_b': np.zeros(D, np.float32),
        'ln2_g': np.ones(D, np.float32), 'ln2_b': np.zeros(D, np.float32),
        'ln3_g': np.ones(D, np.float32), 'ln3_b': np.zeros(D, np.float32),
        'target_h': 32, 'target_w': 32,
    }
    out = kernel(**demo)
    print(out.shape, out.dtype, float(np.abs(out).max()))

